# revision 21
# baseline (speedup 1.0000x reference)
"""ContraNorm Trainium2 kernel: out = 1.2*x - 0.2 * softmax(xn @ xn^T) @ x per batch.

Full input x [8, 2048, 512] f32; batch dim sharded across 8 NeuronCores
(data-parallel, no collectives). Each core runs an identical Bass/Tile program
on its [2048, 512] slice.

Exploits symmetry of sim = xn @ xn^T. Row-chunk orientation: chunk c
(rows 128c..128c+127 on partitions) computes sim columns b >= 128c only
(upper trapezoid, 144/256 blocks). The lower-left blocks are mirrors:
E2[:, j, cP:(c+1)P] = T(E2[:, c, jP:(j+1)P]) for j > c, produced by PE
matmul-transpose (lhsT = E-block, rhs = fp8 identity -> f32 PSUM) plus a
batched cast-copy back to fp8 SBUF. This halves both MM1 PE work and ACT
exp work (the two largest engine costs in the cost-model timeline).

MM1 halves start 512-aligned at/below 128c (a few columns below the
diagonal are computed but never read) so each [P, <=1024] PSUM tile is
drained by ONE exp instruction (ACT per-instruction overhead is ~150ns).

Work is spread across engines (cost-model gantt driven; Pool/GPSIMD cannot
read PSUM, which rules it out for all PSUM->SBUF copies):
  setup: ssq split ACT (Square + accum_out) / DVE (stt + accum_out); sqrt
  in two batches of 8 (halves the barrier; Sqrt and Exp never share an ACT
  table set so batching also bounds table thrash); xn alternates DVE/ACT
  (Copy w/ per-partition scale); xnT copy-out alternates DVE/ACT; xe cast
  (only needed by MM2) runs on Pool.
  main: exp on ACT; mirror copies alternate DVE/ACT; attn scale (po *
  -0.2/D, per-partition scale ptr) and combine on DVE.
  DMA queues: inputs on SP, outputs alternate SP/ACT, so consecutive
  repeat bodies overlap and out-DMA dispatch does not stall exp dispatch.

Pools are created once and shared by all repeat bodies so repeated bodies
pipeline without per-body drains (steady-state measurement matches the
single-body program the grader runs).
"""

import sys

if "/opt/trn_rl_repo" not in sys.path:
    sys.path.insert(0, "/opt/trn_rl_repo")

from contextlib import ExitStack

import numpy as np

import concourse.bass as bass
import concourse.tile as tile
import concourse.mybir as mybir
from concourse import bacc
from concourse.masks import make_identity
from concourse.bass_utils import run_bass_kernel_spmd

F32 = mybir.dt.float32
BF16 = mybir.dt.bfloat16
FP8 = mybir.dt.float8e4
AF = mybir.ActivationFunctionType
ALU = mybir.AluOpType
DR = mybir.MatmulPerfMode.DoubleRow

B = 8
P = 128
N = 2048
D = 512
NT = N // P      # 16 row chunks
DS = D // P      # 4 d subtiles

VARIANT = ""  # debug bisect switches, comma-separated


def make_pools(ctx: ExitStack, tc: tile.TileContext):
    pools = {}
    pools["singles"] = ctx.enter_context(tc.tile_pool(name="singles", bufs=1))
    pools["scratch"] = ctx.enter_context(tc.tile_pool(name="scratch", bufs=3))
    pools["stats"] = ctx.enter_context(tc.tile_pool(name="stats", bufs=8))
    pools["xnpool"] = ctx.enter_context(tc.tile_pool(name="xnpool", bufs=4))
    pools["tmppool"] = ctx.enter_context(tc.tile_pool(name="tmppool", bufs=3))
    pools["opool"] = ctx.enter_context(tc.tile_pool(name="opool", bufs=3))
    # PSUM budget (8 banks): psumS 2x2 (<=1024-wide MM1+exp tiles; ring
    # depth lets MM1 run ahead of the slower exp drain), psumM 2x1 (setup
    # transposes + mirror transposes, shared tag), psumO 1x2 (MM2 numerator +
    # denominator at col 512).
    pools["psumS"] = ctx.enter_context(tc.tile_pool(name="psumS", bufs=2, space="PSUM"))
    pools["psumM"] = ctx.enter_context(tc.tile_pool(name="psumM", bufs=2, space="PSUM"))
    pools["psumO"] = ctx.enter_context(tc.tile_pool(name="psumO", bufs=1, space="PSUM"))
    return pools


def contranorm_body(pools, tc: tile.TileContext, out_ap: bass.AP, x_ap: bass.AP,
                    first: bool):
    nc = tc.nc
    variants = set(VARIANT.split(","))

    singles = pools["singles"]
    scratch = pools["scratch"]
    stats = pools["stats"]
    xnpool = pools["xnpool"]
    tmppool = pools["tmppool"]
    opool = pools["opool"]
    psumS, psumM = pools["psumS"], pools["psumM"]
    psumO = pools["psumO"]

    # persistent tensors (same tiles every repeat; dependency-tracked)
    if first:
        xf = singles.tile([P, NT, D], F32, tag="xf")
        xe = singles.tile([P, NT, D + 16], FP8, tag="xe")
        xnT = singles.tile([P, DS, N], FP8, tag="xnT")
        E2 = singles.tile([P, NT, N], FP8, tag="E2")
        ssqA = singles.tile([P, NT], F32, tag="ssqA")
        nrmA = singles.tile([P, NT], F32, tag="nrmA")
        rnA = singles.tile([P, NT], F32, tag="rnA")
        identB = singles.tile([P, P], BF16, tag="identB")
        identE = singles.tile([P, P], FP8, tag="identE")
        pools.update(xf=xf, xe=xe, xnT=xnT, E2=E2, ssqA=ssqA, nrmA=nrmA,
                     rnA=rnA, identB=identB, identE=identE)
        make_identity(nc, identB)
        make_identity(nc, identE)
        nc.vector.memset(xe[:, :, D:D + 1], 1.0)
    xf, xe, xnT, E2 = pools["xf"], pools["xe"], pools["xnT"], pools["E2"]
    ssqA, nrmA, rnA = pools["ssqA"], pools["nrmA"], pools["rnA"]
    identB, identE = pools["identB"], pools["identE"]

    # ---------------- setup: norms, xn, transpose ----------------
    for i in range(NT):
        nc.sync.dma_start(xf[:, i, :], x_ap[i * P:(i + 1) * P, :])
        if i % 2 == 0:
            # ssq via ACT Square + accum_out
            sq = scratch.tile([P, D], F32, tag="sq")
            nc.scalar.activation(sq, xf[:, i, :], AF.Square,
                                 accum_out=ssqA[:, i:i + 1])
        else:
            # ssq via DVE: sq = (x * 1) * x, accum_out = sum(sq)
            sq = scratch.tile([P, D], F32, tag="sq")
            nc.vector.scalar_tensor_tensor(
                sq, xf[:, i, :], 1.0, xf[:, i, :], op0=ALU.mult, op1=ALU.mult,
                accum_out=ssqA[:, i:i + 1])
        # xe chunk (only needed once MM2 starts): Pool
        nc.gpsimd.tensor_copy(xe[:, i, 0:D], xf[:, i, :])
        if i == 7:
            nc.scalar.activation(nrmA[:, 0:8], ssqA[:, 0:8], AF.Sqrt)
            nc.vector.reciprocal(rnA[:, 0:8], nrmA[:, 0:8])
        elif i == 15:
            nc.scalar.activation(nrmA[:, 8:16], ssqA[:, 8:16], AF.Sqrt)
            nc.vector.reciprocal(rnA[:, 8:16], nrmA[:, 8:16])
    copy_engs = [nc.vector.tensor_copy, nc.scalar.copy]
    for i in range(NT):
        xn = xnpool.tile([P, D], BF16, tag="xn")
        if i % 2 == 0:
            nc.vector.tensor_scalar_mul(xn, xf[:, i, :], rnA[:, i:i + 1])
        else:
            nc.scalar.activation(xn, xf[:, i, :], AF.Copy, scale=rnA[:, i:i + 1])
        pt = psumM.tile([P, DS, P], BF16, tag="pm")
        for dc in range(DS):
            nc.tensor.transpose(pt[:, dc, :], xn[:, dc * P:(dc + 1) * P], identB)
        copy_engs[i % 2](xnT[:, :, i * P:(i + 1) * P], pt)

    # ---------------- main loop: one row-chunk c at a time ----------------
    mir_flip = 0
    for c in range(NT):
        # MM1 + exp over the trapezoid b in [128c, 2048), two 512-banks per
        # PSUM tile. Matmul halves start at 512-aligned a0 <= 128c (the few
        # columns below 128c are computed but never read); exp covers the
        # valid contiguous tail of the tile in one instruction.
        a0 = (128 * c // 512) * 512
        for t0 in range(a0, N, 1024):
            t1 = min(t0 + 1024, N)
            ps = psumS.tile([P, t1 - t0], F32, tag="ps")
            for (s0, s1) in ((t0, min(t0 + 512, t1)), (t0 + 512, t1)):
                if s0 >= s1:
                    continue
                for g in range(2):
                    nc.tensor.matmul(
                        ps[:, s0 - t0:s1 - t0],
                        lhsT=xnT[:, 2 * g:2 * g + 2, c * P:(c + 1) * P],
                        rhs=xnT[:, 2 * g:2 * g + 2, s0:s1],
                        start=(g == 0), stop=(g == 1), perf_mode=DR)
            b0 = max(t0, 128 * c)  # valid region start
            nc.scalar.activation(E2[:, c, b0:t1], ps[:, b0 - t0:], AF.Exp)
            # mirrors for the full blocks inside this tile: j > c
            j0 = max(c + 1, (b0 + P - 1) // P)
            j1 = t1 // P
            for w0 in range(j0, j1, 4):
                w1 = min(w0 + 4, j1)
                nb = w1 - w0
                pm = psumM.tile([P, nb, P], F32, tag="pm")
                for t in range(nb):
                    j = w0 + t
                    nc.tensor.matmul(
                        pm[:, t, :],
                        lhsT=E2[:, c, j * P:(j + 1) * P],
                        rhs=identE, start=True, stop=True)
                if mir_flip % 2 == 0:
                    nc.vector.tensor_copy(E2[:, w0:w1, c * P:(c + 1) * P], pm)
                else:
                    nc.scalar.copy(E2[:, w0:w1, c * P:(c + 1) * P], pm)
                mir_flip += 1

        # MM2 for out row-tile h = c (all needed E2 slices now exist)
        h = c
        po = psumO.tile([P, 1024], F32, tag="po")  # [0:512]=num, [512]=denom
        for g in range(NT // 2):
            lhsT = E2[:, 2 * g:2 * g + 2, h * P:(h + 1) * P]
            nc.tensor.matmul(po[:, 0:D], lhsT, xe[:, 2 * g:2 * g + 2, 0:D],
                             start=(g == 0), stop=(g == NT // 2 - 1), perf_mode=DR)
            nc.tensor.matmul(po[:, D:D + 1], lhsT, xe[:, 2 * g:2 * g + 2, D:D + 1],
                             start=(g == 0), stop=(g == NT // 2 - 1), perf_mode=DR)
        # s = -0.2 / D
        sD = stats.tile([P, 1], F32, tag="sD")
        nc.vector.tensor_scalar_mul(sD, po[:, D:D + 1], -5.0)
        rD = stats.tile([P, 1], F32, tag="rD")
        nc.vector.reciprocal(rD, sD)
        # tmp = O * s (per-partition scale ptr, PSUM -> SBUF), ACT/DVE split
        tmp = tmppool.tile([P, D], F32, tag="tmp")
        nc.vector.tensor_scalar_mul(tmp, po[:, 0:D], rD)
        # out = x * 1.2 + tmp
        ob = opool.tile([P, D], F32, tag="ob")
        nc.vector.scalar_tensor_tensor(
            ob, xf[:, h, :], 1.2, tmp, op0=ALU.mult, op1=ALU.add)
        (nc.sync if h % 2 == 0 else nc.scalar).dma_start(
            out_ap[h * P:(h + 1) * P, :], ob)


def build_nc(repeats: int = 1, loop: int = 0):
    """Build + compile the per-core Bass program. `repeats` re-emits the body
    (sharing pools/SBUF); `loop` wraps the body in a For_i hardware loop --
    both are for steady-state timing measurements."""
    nc = bacc.Bacc("TRN2", target_bir_lowering=False, debug=False, enable_asserts=False)
    x = nc.dram_tensor("x", [N, D], F32, kind="ExternalInput").ap()
    out = nc.dram_tensor("out", [N, D], F32, kind="ExternalOutput").ap()
    with tile.TileContext(nc) as tc:
        with ExitStack() as ctx:
            pools = make_pools(ctx, tc)
            if loop:
                with tc.For_i(0, loop, 1):
                    contranorm_body(pools, tc, out, x, first=True)
            else:
                for r in range(repeats):
                    contranorm_body(pools, tc, out, x, first=(r == 0))
    nc.compile()
    return nc


_nc_cache = {}


def kernel(x: np.ndarray) -> np.ndarray:
    assert x.shape == (B, N, D), x.shape
    x = np.ascontiguousarray(x, dtype=np.float32)
    if "nc" not in _nc_cache:
        _nc_cache["nc"] = build_nc()
    nc = _nc_cache["nc"]
    in_maps = [{"x": x[i]} for i in range(B)]
    res = run_bass_kernel_spmd(nc, in_maps, core_ids=list(range(B)))
    return np.stack([r["out"] for r in res.results], axis=0)


# revision 22
# speedup vs baseline: 1.1640x; 1.1640x over previous
"""ContraNorm Trainium2 kernel: out = 1.2*x - 0.2 * softmax(xn @ xn^T) @ x per batch.

Full input x [8, 2048, 512] f32; batch dim sharded across 8 NeuronCores
(data-parallel, no collectives). Each core runs an identical Bass/Tile program
on its [2048, 512] slice.

Exploits symmetry of sim = xn @ xn^T. Row-chunk orientation: chunk c
(rows 128c..128c+127 on partitions) computes sim columns b >= 128c only
(upper trapezoid, 144/256 blocks). The lower-left blocks are mirrors:
E2[:, j, cP:(c+1)P] = T(E2[:, c, jP:(j+1)P]) for j > c, produced by PE
matmul-transpose (lhsT = E-block, rhs = fp8 identity -> f32 PSUM) plus a
batched cast-copy back to fp8 SBUF. This halves both MM1 PE work and ACT
exp work (the two largest engine costs in the cost-model timeline).

MM1 halves start 512-aligned at/below 128c (a few columns below the
diagonal are computed but never read) so each [P, <=1024] PSUM tile is
drained by ONE exp instruction (ACT per-instruction overhead is ~150ns).

Work is spread across engines (cost-model gantt driven; Pool/GPSIMD cannot
read PSUM, which rules it out for all PSUM->SBUF copies):
  setup: ssq split ACT (Square + accum_out) / DVE (stt + accum_out); sqrt
  in two batches of 8 (halves the barrier; Sqrt and Exp never share an ACT
  table set so batching also bounds table thrash); xn alternates DVE/ACT
  (Copy w/ per-partition scale); xnT copy-out alternates DVE/ACT; xe cast
  (only needed by MM2) runs on Pool.
  main: exp on ACT; mirror copies alternate DVE/ACT; attn scale (po *
  -0.2/D, per-partition scale ptr) and combine on DVE.
  DMA queues: inputs on SP, outputs alternate SP/ACT, so consecutive
  repeat bodies overlap and out-DMA dispatch does not stall exp dispatch.

Pools are created once and shared by all repeat bodies so repeated bodies
pipeline without per-body drains (steady-state measurement matches the
single-body program the grader runs).
"""

import sys

if "/opt/trn_rl_repo" not in sys.path:
    sys.path.insert(0, "/opt/trn_rl_repo")

from contextlib import ExitStack

import numpy as np

import concourse.bass as bass
import concourse.tile as tile
import concourse.mybir as mybir
from concourse import bacc
from concourse.masks import make_identity
from concourse.bass_utils import run_bass_kernel_spmd

F32 = mybir.dt.float32
BF16 = mybir.dt.bfloat16
FP8 = mybir.dt.float8e4
AF = mybir.ActivationFunctionType
ALU = mybir.AluOpType
DR = mybir.MatmulPerfMode.DoubleRow

B = 8
P = 128
N = 2048
D = 512
NT = N // P      # 16 row chunks
DS = D // P      # 4 d subtiles

VARIANT = ""  # debug bisect switches, comma-separated


def make_pools(ctx: ExitStack, tc: tile.TileContext):
    pools = {}
    pools["singles"] = ctx.enter_context(tc.tile_pool(name="singles", bufs=1))
    pools["scratch"] = ctx.enter_context(tc.tile_pool(name="scratch", bufs=3))
    pools["stats"] = ctx.enter_context(tc.tile_pool(name="stats", bufs=8))
    pools["xnpool"] = ctx.enter_context(tc.tile_pool(name="xnpool", bufs=4))
    pools["tmppool"] = ctx.enter_context(tc.tile_pool(name="tmppool", bufs=3))
    pools["opool"] = ctx.enter_context(tc.tile_pool(name="opool", bufs=3))
    # PSUM budget (8 banks): psumS 2x2 (<=1024-wide MM1+exp tiles; ring
    # depth lets MM1 run ahead of the slower exp drain), psumM 2x1 (setup
    # transposes + mirror transposes, shared tag), psumO 1x2 (MM2 numerator +
    # denominator at col 512).
    pools["psumS"] = ctx.enter_context(tc.tile_pool(name="psumS", bufs=2, space="PSUM"))
    pools["psumM"] = ctx.enter_context(tc.tile_pool(name="psumM", bufs=2, space="PSUM"))
    pools["psumO"] = ctx.enter_context(tc.tile_pool(name="psumO", bufs=1, space="PSUM"))
    return pools


def contranorm_body(pools, tc: tile.TileContext, out_ap: bass.AP, x_ap: bass.AP,
                    first: bool):
    nc = tc.nc
    variants = set(VARIANT.split(","))

    singles = pools["singles"]
    scratch = pools["scratch"]
    stats = pools["stats"]
    xnpool = pools["xnpool"]
    tmppool = pools["tmppool"]
    opool = pools["opool"]
    psumS, psumM = pools["psumS"], pools["psumM"]
    psumO = pools["psumO"]

    # persistent tensors (same tiles every repeat; dependency-tracked)
    if first:
        xf = singles.tile([P, NT, D], F32, tag="xf")
        xe = singles.tile([P, NT, D + 16], FP8, tag="xe")
        xnT = singles.tile([P, DS, N], FP8, tag="xnT")
        E2 = singles.tile([P, NT, N], FP8, tag="E2")
        ssqA = singles.tile([P, NT], F32, tag="ssqA")
        nrmA = singles.tile([P, NT], F32, tag="nrmA")
        rnA = singles.tile([P, NT], F32, tag="rnA")
        identB = singles.tile([P, P], BF16, tag="identB")
        identE = singles.tile([P, P], FP8, tag="identE")
        pools.update(xf=xf, xe=xe, xnT=xnT, E2=E2, ssqA=ssqA, nrmA=nrmA,
                     rnA=rnA, identB=identB, identE=identE)
        make_identity(nc, identB)
        make_identity(nc, identE)
        nc.vector.memset(xe[:, :, D:D + 1], 1.0)
    xf, xe, xnT, E2 = pools["xf"], pools["xe"], pools["xnT"], pools["E2"]
    ssqA, nrmA, rnA = pools["ssqA"], pools["nrmA"], pools["rnA"]
    identB, identE = pools["identB"], pools["identE"]

    # ---------------- setup: norms, xn, transpose ----------------
    for i in range(NT):
        nc.sync.dma_start(xf[:, i, :], x_ap[i * P:(i + 1) * P, :])
        if i % 2 == 0:
            # ssq via ACT Square + accum_out
            sq = scratch.tile([P, D], F32, tag="sq")
            nc.scalar.activation(sq, xf[:, i, :], AF.Square,
                                 accum_out=ssqA[:, i:i + 1])
        else:
            # ssq via DVE: sq = (x * 1) * x, accum_out = sum(sq)
            sq = scratch.tile([P, D], F32, tag="sq")
            nc.vector.scalar_tensor_tensor(
                sq, xf[:, i, :], 1.0, xf[:, i, :], op0=ALU.mult, op1=ALU.mult,
                accum_out=ssqA[:, i:i + 1])
        # xe chunk (only needed once MM2 starts): Pool
        nc.gpsimd.tensor_copy(xe[:, i, 0:D], xf[:, i, :])
        if i == 7:
            nc.scalar.activation(nrmA[:, 0:8], ssqA[:, 0:8], AF.Sqrt)
            nc.vector.reciprocal(rnA[:, 0:8], nrmA[:, 0:8])
        elif i == 15:
            nc.scalar.activation(nrmA[:, 8:16], ssqA[:, 8:16], AF.Sqrt)
            nc.vector.reciprocal(rnA[:, 8:16], nrmA[:, 8:16])
    copy_engs = [nc.vector.tensor_copy, nc.scalar.copy]
    for i in range(NT):
        xn = xnpool.tile([P, D], BF16, tag="xn")
        if i % 2 == 0:
            nc.vector.tensor_scalar_mul(xn, xf[:, i, :], rnA[:, i:i + 1])
        else:
            nc.scalar.activation(xn, xf[:, i, :], AF.Copy, scale=rnA[:, i:i + 1])
        pt = psumM.tile([P, DS, P], BF16, tag="pm")
        for dc in range(DS):
            nc.tensor.transpose(pt[:, dc, :], xn[:, dc * P:(dc + 1) * P], identB)
        copy_engs[i % 2](xnT[:, :, i * P:(i + 1) * P], pt)

    # ---------------- main loop: one row-chunk c at a time -------------
    # Software-pipelined by one chunk: MM2 for chunk c-1 (whose E2 slices
    # completed during chunk c-1's mirror copies) is emitted BEFORE chunk
    # c's MM1, so the in-order PE queue always has ready work at its head
    # instead of stalling on mirror-transposes waiting for the exp drain.
    def emit_mm2(h):
        po = psumO.tile([P, 1024], F32, tag="po")  # [0:512]=num, [512]=denom
        for g in range(NT // 2):
            lhsT = E2[:, 2 * g:2 * g + 2, h * P:(h + 1) * P]
            nc.tensor.matmul(po[:, 0:D], lhsT, xe[:, 2 * g:2 * g + 2, 0:D],
                             start=(g == 0), stop=(g == NT // 2 - 1), perf_mode=DR)
            nc.tensor.matmul(po[:, D:D + 1], lhsT, xe[:, 2 * g:2 * g + 2, D:D + 1],
                             start=(g == 0), stop=(g == NT // 2 - 1), perf_mode=DR)
        # s = -0.2 / D
        sD = stats.tile([P, 1], F32, tag="sD")
        nc.vector.tensor_scalar_mul(sD, po[:, D:D + 1], -5.0)
        rD = stats.tile([P, 1], F32, tag="rD")
        nc.vector.reciprocal(rD, sD)
        # tmp = O * s (per-partition scale ptr, PSUM -> SBUF)
        tmp = tmppool.tile([P, D], F32, tag="tmp")
        nc.vector.tensor_scalar_mul(tmp, po[:, 0:D], rD)
        # out = x * 1.2 + tmp
        ob = opool.tile([P, D], F32, tag="ob")
        nc.vector.scalar_tensor_tensor(
            ob, xf[:, h, :], 1.2, tmp, op0=ALU.mult, op1=ALU.add)
        (nc.sync if h % 2 == 0 else nc.scalar).dma_start(
            out_ap[h * P:(h + 1) * P, :], ob)

    mir_flip = 0
    for c in range(NT):
        if c > 0:
            emit_mm2(c - 1)
        # MM1 + exp over the trapezoid b in [128c, 2048), two 512-banks per
        # PSUM tile. Matmul halves start at 512-aligned a0 <= 128c (the few
        # columns below 128c are computed but never read); exp covers the
        # valid contiguous tail of the tile in one instruction.
        a0 = (128 * c // 512) * 512
        for t0 in range(a0, N, 1024):
            t1 = min(t0 + 1024, N)
            ps = psumS.tile([P, t1 - t0], F32, tag="ps")
            for (s0, s1) in ((t0, min(t0 + 512, t1)), (t0 + 512, t1)):
                if s0 >= s1:
                    continue
                for g in range(2):
                    nc.tensor.matmul(
                        ps[:, s0 - t0:s1 - t0],
                        lhsT=xnT[:, 2 * g:2 * g + 2, c * P:(c + 1) * P],
                        rhs=xnT[:, 2 * g:2 * g + 2, s0:s1],
                        start=(g == 0), stop=(g == 1), perf_mode=DR)
            b0 = max(t0, 128 * c)  # valid region start
            nc.scalar.activation(E2[:, c, b0:t1], ps[:, b0 - t0:], AF.Exp)
            # mirrors for the full blocks inside this tile: j > c
            j0 = max(c + 1, (b0 + P - 1) // P)
            j1 = t1 // P
            for w0 in range(j0, j1, 4):
                w1 = min(w0 + 4, j1)
                nb = w1 - w0
                pm = psumM.tile([P, nb, P], F32, tag="pm")
                for t in range(nb):
                    j = w0 + t
                    nc.tensor.matmul(
                        pm[:, t, :],
                        lhsT=E2[:, c, j * P:(j + 1) * P],
                        rhs=identE, start=True, stop=True)
                if mir_flip % 2 == 0:
                    nc.vector.tensor_copy(E2[:, w0:w1, c * P:(c + 1) * P], pm)
                else:
                    nc.scalar.copy(E2[:, w0:w1, c * P:(c + 1) * P], pm)
                mir_flip += 1

    emit_mm2(NT - 1)


def build_nc(repeats: int = 1, loop: int = 0):
    """Build + compile the per-core Bass program. `repeats` re-emits the body
    (sharing pools/SBUF); `loop` wraps the body in a For_i hardware loop --
    both are for steady-state timing measurements."""
    nc = bacc.Bacc("TRN2", target_bir_lowering=False, debug=False, enable_asserts=False)
    x = nc.dram_tensor("x", [N, D], F32, kind="ExternalInput").ap()
    out = nc.dram_tensor("out", [N, D], F32, kind="ExternalOutput").ap()
    with tile.TileContext(nc) as tc:
        with ExitStack() as ctx:
            pools = make_pools(ctx, tc)
            if loop:
                with tc.For_i(0, loop, 1):
                    contranorm_body(pools, tc, out, x, first=True)
            else:
                for r in range(repeats):
                    contranorm_body(pools, tc, out, x, first=(r == 0))
    nc.compile()
    return nc


_nc_cache = {}


def kernel(x: np.ndarray) -> np.ndarray:
    assert x.shape == (B, N, D), x.shape
    x = np.ascontiguousarray(x, dtype=np.float32)
    if "nc" not in _nc_cache:
        _nc_cache["nc"] = build_nc()
    nc = _nc_cache["nc"]
    in_maps = [{"x": x[i]} for i in range(B)]
    res = run_bass_kernel_spmd(nc, in_maps, core_ids=list(range(B)))
    return np.stack([r["out"] for r in res.results], axis=0)
